# revision 1
# baseline (speedup 1.0000x reference)
"""HNLoRALinear Trainium2 kernel.

out[b,s,o] = x[b] @ W^T + bias + SCALE * (x[b] @ A[b]) @ B[b]

Sharding: 8 cores = 4 batches x 2 sequence-halves. Each core computes
its [1024 tokens, 4096 outs] output block, TRANSPOSED on device
(outs on PSUM partitions, tokens as the moving dim) so that:
  - the stationary matmul operand is a [128, 128] W^T chunk,
  - the moving operand is a 512-token slice of the SBUF-resident x^T
    (N=512 stream time ~213ns fully hides the ~197ns 4-byte f32r
    weight load; at N=256 the load was exposed),
  - consecutive k-chunk matmuls accumulate in PSUM.
The host pre-transposes x and W so the contraction dim (D_IN) lands on
SBUF partitions with no on-device transposes, and un-transposes each
core's [4096, 1024] output block when assembling the result.

The per-sample LoRA correction + bias ride along as one extra K=17
matmul per output tile: [SCALE*B ; bias]^T-chunk (stationary) @
[low ; ones] (moving), accumulated into the same PSUM group.

Matmuls run in float32r (TF32-like, full PE rate at moving-dim>=256,
~1.2e-4 rel err end-to-end).
"""
import numpy as np

import concourse.bass as bass  # noqa: F401  (bass must import before tile)
import concourse.mybir as mybir
import concourse.tile as tile
from concourse import bacc
from concourse.bass_utils import run_bass_kernel_spmd

# Problem shapes (hardcoded per contract).
B, S, D_IN, D_OUT, R = 4, 2048, 4096, 4096, 16
XG = 8                 # x DMA groups (separate tiles so deps are per-group)
SCALE = 32.0 / 16.0
SH = S // 2            # tokens per core
P = 128
KC = D_IN // P         # 32 contraction chunks
O_CHUNKS = D_OUT // P  # 32 output-feature chunks (PSUM partition dim)
TN = 512               # moving-dim token group width
TGROUPS = SH // TN     # 2
KG = KC // XG          # k-chunks per x group
RA = R + 1             # augmented rank (lora + bias row)

_cached_nc = None


def _build():
    f32r = mybir.dt.float32r
    f32 = mybir.dt.float32
    nc = bacc.Bacc(
        "TRN2", target_bir_lowering=False, debug=False, enable_asserts=False
    )
    xt = nc.dram_tensor("xt", [XG, P, KG * SH], f32r, kind="ExternalInput")
    wt = nc.dram_tensor("wt", [O_CHUNKS, P, KC * P], f32r, kind="ExternalInput")
    apk = nc.dram_tensor("apack", [P, KC * R], f32r, kind="ExternalInput")
    bga = nc.dram_tensor("baug", [RA, D_OUT], f32r, kind="ExternalInput")
    ot_d = nc.dram_tensor("ot", [D_OUT, SH], f32, kind="ExternalOutput")

    with tile.TileContext(nc) as tc:
        with (
            tc.tile_pool(name="xp", bufs=1) as xp,
            tc.tile_pool(name="wp", bufs=2) as wp,
            tc.tile_pool(name="cp", bufs=1) as cp,
            tc.tile_pool(name="op", bufs=3) as op,
            tc.tile_pool(name="pp", bufs=4, space="PSUM") as pp,
            tc.tile_pool(name="lp", bufs=2, space="PSUM") as lp,
        ):
            at = cp.tile([P, KC * R], f32r, name="at")
            nc.sync.dma_start(out=at[:], in_=apk.ap())
            bt = cp.tile([RA, D_OUT], f32r, name="bt")
            nc.sync.dma_start(out=bt[:], in_=bga.ap())

            def load_w_strip(o):
                # One fully-contiguous 2D DMA per strip (host pre-packs W
                # as [o_chunk, partition, k*128+c]) -- 3D patterns cost one
                # DMA descriptor per (partition, k) and dispatch ~20x slower.
                wk = wp.tile([P, KC * P], f32r, name="wk")
                nc.sync.dma_start(out=wk[:], in_=wt.ap()[o])
                return wk

            # x^T fully resident as XG separate [128, KG, 1024] tiles (one
            # DMA each) so each matmul depends only on the chunk-group it
            # reads; interleave the first W strips so main o=0/1 can start
            # early.
            xgs = []
            w_strips = {}
            for g in range(XG):
                xg = xp.tile([P, KG * SH], f32r, name=f"xg{g}", tag=f"xg{g}")
                nc.sync.dma_start(out=xg[:], in_=xt.ap()[g])
                xgs.append(xg)
                if g < 2:
                    w_strips[g] = load_w_strip(g)

            def xsl(k, t):
                return xgs[k // KG][:, (k % KG) * SH + t * TN : (k % KG) * SH + (t + 1) * TN]

            # Augmented low-rank activations: rows 0..15 = (x @ A)^T,
            # row 16 = 1. (memset must start at a 32-aligned partition, so
            # fill all 17 rows; the copies below overwrite rows 0..15.)
            low = cp.tile([RA, SH], f32r, name="low")
            nc.gpsimd.memset(low[:].bitcast(f32), 1.0)
            pls = [lp.tile([R, TN], f32, name="pl") for _ in range(TGROUPS)]
            for k in range(KC):
                for t in range(TGROUPS):
                    nc.tensor.matmul(
                        pls[t][:],
                        at[:, k * R : (k + 1) * R],
                        xsl(k, t),
                        start=(k == 0),
                        stop=(k == KC - 1),
                    )
            for t in range(TGROUPS):
                nc.vector.tensor_copy(low[0:R, t * TN : (t + 1) * TN], pls[t][:])

            for o in range(O_CHUNKS):
                wk = w_strips.pop(o) if o in w_strips else load_w_strip(o)
                if o + 1 < O_CHUNKS and o >= 1 and (o + 1) not in w_strips:
                    w_strips[o + 1] = load_w_strip(o + 1)
                otile = op.tile([P, SH], f32, name="otile")
                for t in range(TGROUPS):
                    ps = pp.tile([P, TN], f32, name="ps")
                    for k in range(KC):
                        nc.tensor.matmul(
                            ps[:],
                            wk[:, k * P : (k + 1) * P],
                            xsl(k, t),
                            start=(k == 0),
                            stop=False,
                        )
                    nc.tensor.matmul(
                        ps[:],
                        bt[:, o * P : (o + 1) * P],
                        low[:, t * TN : (t + 1) * TN],
                        start=False,
                        stop=True,
                    )
                    nc.vector.tensor_copy(otile[:, t * TN : (t + 1) * TN], ps[:])
                nc.sync.dma_start(out=ot_d.ap()[o * P : (o + 1) * P, :], in_=otile[:])
    nc.compile()
    return nc


def _get_nc():
    global _cached_nc
    if _cached_nc is None:
        _cached_nc = _build()
    return _cached_nc


def _in_maps(x, weight, bias, lora_A, lora_B):
    # W^T packed as [o_chunk, partition, k*128+c]: element (o*128+c, k*128+p)
    # of W -> wt[o, p, k*128+c]; shared by all cores.
    wt = np.ascontiguousarray(
        weight.T.reshape(KC, P, O_CHUNKS, P).transpose(2, 1, 0, 3).reshape(
            O_CHUNKS, P, KC * P
        )
    ).astype(np.float32, copy=False)
    bias = bias.astype(np.float32, copy=False)
    maps = []
    for c in range(8):
        b, h = divmod(c, 2)
        xtc = np.ascontiguousarray(
            x[b, h * SH : (h + 1) * SH, :].T.reshape(XG, KG, P, SH)
            .transpose(0, 2, 1, 3)
            .reshape(XG, P, KG * SH)
        ).astype(np.float32, copy=False)
        apk = np.ascontiguousarray(
            lora_A[b].reshape(KC, P, R).transpose(1, 0, 2).reshape(P, KC * R)
        ).astype(np.float32, copy=False)
        baug = np.concatenate(
            [lora_B[b].astype(np.float32) * np.float32(SCALE), bias[None, :]], axis=0
        )
        maps.append({"xt": xtc, "wt": wt, "apack": apk, "baug": baug})
    return maps


def kernel(x, weight, bias, lora_A, lora_B, _trace=False, _tmpdir=None):
    x = np.asarray(x, dtype=np.float32)
    weight = np.asarray(weight, dtype=np.float32)
    bias = np.asarray(bias, dtype=np.float32)
    lora_A = np.asarray(lora_A, dtype=np.float32)
    lora_B = np.asarray(lora_B, dtype=np.float32)

    nc = _get_nc()
    maps = _in_maps(x, weight, bias, lora_A, lora_B)
    res = run_bass_kernel_spmd(
        nc, maps, list(range(8)), trace=_trace, tmpdir=_tmpdir
    )
    out = np.empty((B, S, D_OUT), np.float32)
    for c in range(8):
        b, h = divmod(c, 2)
        out[b, h * SH : (h + 1) * SH, :] = res.results[c]["ot"].T
    if _trace:
        return out, res
    return out



# revision 2
# speedup vs baseline: 1.1238x; 1.1238x over previous
"""HNLoRALinear Trainium2 kernel (bf16 v2).

out[b,s,o] = x[b] @ W^T + bias + SCALE * (x[b] @ A[b]) @ B[b]

Sharding: 8 cores = 4 batches x 2 sequence-halves. Each core computes
its [1024 tokens, 4096 outs] output block, TRANSPOSED on device
(outs on PSUM partitions, tokens as the moving dim) so that:
  - the stationary matmul operand is a [128, 128] W^T chunk,
  - the moving operand is a 512-token slice of the SBUF-resident x^T,
  - consecutive k-chunk matmuls accumulate in fp32 PSUM.
All inputs are cast to bf16 on the host: the PE streams 1 column/cycle
for bf16 and f32r alike, but bf16 halves DMA bytes (84->42 MB/core),
halves SBUF pressure, and enables Fast Weight Load so the 128-col
LDWEIGHTS (~93ns) fully hides under the 512-col stream (~213ns).
Accumulation stays fp32 in PSUM; rel err ~2e-3 vs the 2e-2 budget.

The per-sample LoRA correction + bias ride along as one extra K=17
matmul per output tile: [SCALE*B ; bias]^T-chunk (stationary) @
[low ; ones] (moving), accumulated into the same PSUM group.
"""
import numpy as np
import ml_dtypes

import concourse.bass as bass  # noqa: F401  (bass must import before tile)
import concourse.mybir as mybir
import concourse.tile as tile
from concourse import bacc
from concourse.bass_utils import run_bass_kernel_spmd

# Problem shapes (hardcoded per contract).
B, S, D_IN, D_OUT, R = 4, 2048, 4096, 4096, 16
XG = 8                 # x DMA groups (separate tiles so deps are per-group)
SCALE = 32.0 / 16.0
SH = S // 2            # tokens per core
P = 128
KC = D_IN // P         # 32 contraction chunks
O_CHUNKS = D_OUT // P  # 32 output-feature chunks (PSUM partition dim)
TN = 512               # moving-dim token group width
TGROUPS = SH // TN     # 2
KG = KC // XG          # k-chunks per x group
RA = R + 1             # augmented rank (lora + bias row)

BF16 = ml_dtypes.bfloat16

_cached_nc = None


def _build():
    bf16 = mybir.dt.bfloat16
    f32 = mybir.dt.float32
    nc = bacc.Bacc(
        "TRN2", target_bir_lowering=False, debug=False, enable_asserts=False
    )
    xt = nc.dram_tensor("xt", [XG, P, KG * SH], bf16, kind="ExternalInput")
    wt = nc.dram_tensor("wt", [O_CHUNKS, P, KC * P], bf16, kind="ExternalInput")
    apk = nc.dram_tensor("apack", [P, KC * R], bf16, kind="ExternalInput")
    bga = nc.dram_tensor("baug", [RA, D_OUT], bf16, kind="ExternalInput")
    ot_d = nc.dram_tensor("ot", [D_OUT, SH], f32, kind="ExternalOutput")

    with tile.TileContext(nc) as tc:
        with (
            tc.tile_pool(name="xp", bufs=1) as xp,
            tc.tile_pool(name="wp", bufs=3) as wp,
            tc.tile_pool(name="cp", bufs=1) as cp,
            tc.tile_pool(name="op", bufs=3) as op,
            tc.tile_pool(name="pp", bufs=4, space="PSUM") as pp,
            tc.tile_pool(name="lp", bufs=2, space="PSUM") as lp,
        ):
            at = cp.tile([P, KC * R], bf16, name="at")
            nc.sync.dma_start(out=at[:], in_=apk.ap())

            def load_w_strip(o):
                # One fully-contiguous 2D DMA per strip (host pre-packs W
                # as [o_chunk, partition, k*128+c]) -- 3D patterns cost one
                # DMA descriptor per (partition, k) and dispatch ~20x slower.
                wk = wp.tile([P, KC * P], bf16, name="wk")
                nc.sync.dma_start(out=wk[:], in_=wt.ap()[o])
                return wk

            # x^T fully resident as XG separate [128, KG, 1024] tiles (one
            # DMA each) so each matmul depends only on the chunk-group it
            # reads; interleave the first W strips so main o=0/1 can start
            # early. bt (lora_B+bias) is only needed at the first augmented
            # matmul, so it is enqueued after xg1.
            xgs = []
            w_strips = {}
            bt = cp.tile([RA, D_OUT], bf16, name="bt")
            for g in range(XG):
                xg = xp.tile([P, KG * SH], bf16, name=f"xg{g}", tag=f"xg{g}")
                nc.sync.dma_start(out=xg[:], in_=xt.ap()[g])
                xgs.append(xg)
                if g < 2:
                    w_strips[g] = load_w_strip(g)
                if g == 1:
                    nc.sync.dma_start(out=bt[:], in_=bga.ap())

            def xsl(k, t):
                return xgs[k // KG][:, (k % KG) * SH + t * TN : (k % KG) * SH + (t + 1) * TN]

            # Augmented low-rank activations: rows 0..15 = (x @ A)^T,
            # row 16 = 1. (memset must start at a 32-aligned partition, so
            # fill all 17 rows; the copies below overwrite rows 0..15.)
            low = cp.tile([RA, SH], bf16, name="low")
            nc.gpsimd.memset(low[:], 1.0)
            pls = [lp.tile([R, TN], f32, name="pl") for _ in range(TGROUPS)]
            for k in range(KC):
                for t in range(TGROUPS):
                    nc.tensor.matmul(
                        pls[t][:],
                        at[:, k * R : (k + 1) * R],
                        xsl(k, t),
                        start=(k == 0),
                        stop=(k == KC - 1),
                    )
            for t in range(TGROUPS):
                nc.vector.tensor_copy(low[0:R, t * TN : (t + 1) * TN], pls[t][:])

            for o in range(O_CHUNKS):
                wk = w_strips.pop(o) if o in w_strips else load_w_strip(o)
                if o + 1 < O_CHUNKS and o >= 1 and (o + 1) not in w_strips:
                    w_strips[o + 1] = load_w_strip(o + 1)
                otile = op.tile([P, SH], f32, name="otile")
                for t in range(TGROUPS):
                    ps = pp.tile([P, TN], f32, name="ps")
                    for k in range(KC):
                        nc.tensor.matmul(
                            ps[:],
                            wk[:, k * P : (k + 1) * P],
                            xsl(k, t),
                            start=(k == 0),
                            stop=False,
                        )
                    nc.tensor.matmul(
                        ps[:],
                        bt[:, o * P : (o + 1) * P],
                        low[:, t * TN : (t + 1) * TN],
                        start=False,
                        stop=True,
                    )
                    nc.vector.tensor_copy(otile[:, t * TN : (t + 1) * TN], ps[:])
                nc.sync.dma_start(out=ot_d.ap()[o * P : (o + 1) * P, :], in_=otile[:])
    nc.compile()
    return nc


def _get_nc():
    global _cached_nc
    if _cached_nc is None:
        _cached_nc = _build()
    return _cached_nc


def _in_maps(x, weight, bias, lora_A, lora_B):
    # W^T packed as [o_chunk, partition, k*128+c]: element (o*128+c, k*128+p)
    # of W -> wt[o, p, k*128+c]; shared by all cores.
    wt = np.ascontiguousarray(
        weight.T.reshape(KC, P, O_CHUNKS, P).transpose(2, 1, 0, 3).reshape(
            O_CHUNKS, P, KC * P
        )
    ).astype(BF16)
    maps = []
    for c in range(8):
        b, h = divmod(c, 2)
        xtc = np.ascontiguousarray(
            x[b, h * SH : (h + 1) * SH, :].T.reshape(XG, KG, P, SH)
            .transpose(0, 2, 1, 3)
            .reshape(XG, P, KG * SH)
        ).astype(BF16)
        apk = np.ascontiguousarray(
            lora_A[b].reshape(KC, P, R).transpose(1, 0, 2).reshape(P, KC * R)
        ).astype(BF16)
        baug = np.concatenate(
            [lora_B[b].astype(np.float32) * np.float32(SCALE), bias[None, :]], axis=0
        ).astype(BF16)
        maps.append({"xt": xtc, "wt": wt, "apack": apk, "baug": baug})
    return maps


def kernel(x, weight, bias, lora_A, lora_B, _trace=False, _tmpdir=None):
    x = np.asarray(x, dtype=np.float32)
    weight = np.asarray(weight, dtype=np.float32)
    bias = np.asarray(bias, dtype=np.float32)
    lora_A = np.asarray(lora_A, dtype=np.float32)
    lora_B = np.asarray(lora_B, dtype=np.float32)

    nc = _get_nc()
    maps = _in_maps(x, weight, bias, lora_A, lora_B)
    res = run_bass_kernel_spmd(
        nc, maps, list(range(8)), trace=_trace, tmpdir=_tmpdir
    )
    out = np.empty((B, S, D_OUT), np.float32)
    for c in range(8):
        b, h = divmod(c, 2)
        out[b, h * SH : (h + 1) * SH, :] = res.results[c]["ot"].T
    if _trace:
        return out, res
    return out


# revision 3
# speedup vs baseline: 1.1710x; 1.0419x over previous
"""HNLoRALinear Trainium2 kernel (bf16 v3).

out[b,s,o] = x[b] @ W^T + bias + SCALE * (x[b] @ A[b]) @ B[b]

Sharding: 8 cores = 4 batches x 2 sequence-halves. Each core computes
its [1024 tokens, 4096 outs] output block, TRANSPOSED on device
(outs on PSUM partitions, tokens as the moving dim) so that:
  - the stationary matmul operand is a [128, 128] W^T chunk,
  - the moving operand is a 512-token slice of the SBUF-resident x^T,
  - consecutive k-chunk matmuls accumulate in fp32 PSUM.
All inputs are cast to bf16 on the host: the PE streams 1 column/cycle
for bf16 and f32r alike, but bf16 halves DMA bytes, halves SBUF
pressure, and enables Fast Weight Load so the 128-col LDWEIGHTS
(~97ns) fully hides under the 512-col stream (~216ns).

The per-sample LoRA correction + bias ride along as one extra matmul
per output tile, with the stationary padded to the full K=128:
rows 0..15 = SCALE*B chunk, row 16 = bias, rows 17..127 = 0 (so the
matching `low` rows can hold anything finite). Full-K stationaries
keep the LDWEIGHTS pull-ahead pipeline intact across the tile
boundary -- a K=17 stationary cost ~200ns of serialized LDW per tile.

Startup: 16 dummy matmuls on a memset scratch tile warm the PE
p-state/HAM while the first DMAs land, and o=0's k-matmuls are
interleaved into the x@A loop on the same per-x-group cadence so the
DMA-paced phase retires real work.
"""
import numpy as np
import ml_dtypes

import concourse.bass as bass  # noqa: F401  (bass must import before tile)
import concourse.mybir as mybir
import concourse.tile as tile
from concourse import bacc
from concourse.bass_utils import run_bass_kernel_spmd

# Problem shapes (hardcoded per contract).
B, S, D_IN, D_OUT, R = 4, 2048, 4096, 4096, 16
XG = 8                 # x DMA groups (separate tiles so deps are per-group)
SCALE = 32.0 / 16.0
SH = S // 2            # tokens per core
P = 128
KC = D_IN // P         # 32 contraction chunks
O_CHUNKS = D_OUT // P  # 32 output-feature chunks (PSUM partition dim)
TN = 512               # moving-dim token group width
TGROUPS = SH // TN     # 2
KG = KC // XG          # k-chunks per x group
RA = R + 1             # augmented rank (lora + bias row)
N_WARM = 16            # PE warm-up dummy matmuls

BF16 = ml_dtypes.bfloat16

_cached_nc = None


def _build():
    bf16 = mybir.dt.bfloat16
    f32 = mybir.dt.float32
    nc = bacc.Bacc(
        "TRN2", target_bir_lowering=False, debug=False, enable_asserts=False
    )
    xt = nc.dram_tensor("xt", [XG, P, KG * SH], bf16, kind="ExternalInput")
    wt = nc.dram_tensor("wt", [O_CHUNKS, P, KC * P], bf16, kind="ExternalInput")
    apk = nc.dram_tensor("apack", [P, KC * R], bf16, kind="ExternalInput")
    bga = nc.dram_tensor("baug", [P, D_OUT], bf16, kind="ExternalInput")
    ot_d = nc.dram_tensor("ot", [D_OUT, SH], f32, kind="ExternalOutput")

    with tile.TileContext(nc) as tc:
        with (
            tc.tile_pool(name="xp", bufs=1) as xp,
            tc.tile_pool(name="wp", bufs=3) as wp,
            tc.tile_pool(name="cp", bufs=1) as cp,
            tc.tile_pool(name="op", bufs=3) as op,
            tc.tile_pool(name="pp", bufs=4, space="PSUM") as pp,
            tc.tile_pool(name="lp", bufs=2, space="PSUM") as lp,
            tc.tile_pool(name="sp", bufs=1, space="PSUM") as sp,
        ):
            at = cp.tile([P, KC * R], bf16, name="at")
            nc.sync.dma_start(out=at[:], in_=apk.ap())

            def load_w_strip(o):
                # One fully-contiguous 2D DMA per strip (host pre-packs W
                # as [o_chunk, partition, k*128+c]) -- 3D patterns cost one
                # DMA descriptor per (partition, k) and dispatch ~20x slower.
                wk = wp.tile([P, KC * P], bf16, name="wk")
                nc.sync.dma_start(out=wk[:], in_=wt.ap()[o])
                return wk

            # x^T fully resident as XG separate [128, KG, 1024] tiles (one
            # DMA each) so each matmul depends only on the chunk-group it
            # reads; interleave the first W strips so main o=0/1 can start
            # early. bt (padded lora_B+bias) is only needed at the first
            # augmented matmul, so it is enqueued after xg1.
            xgs = []
            w_strips = {}
            bt = cp.tile([P, D_OUT], bf16, name="bt")
            for g in range(XG):
                xg = xp.tile([P, KG * SH], bf16, name=f"xg{g}", tag=f"xg{g}")
                nc.sync.dma_start(out=xg[:], in_=xt.ap()[g])
                xgs.append(xg)
                if g < 2:
                    w_strips[g] = load_w_strip(g)
                if g == 1:
                    nc.sync.dma_start(out=bt[:], in_=bga.ap())

            def xsl(k, t):
                return xgs[k // KG][:, (k % KG) * SH + t * TN : (k % KG) * SH + (t + 1) * TN]

            # PE warm-up: dummy matmuls on a zeroed scratch tile with no DMA
            # dependency, so the PE ramps to full clock while at/xg0 land.
            scr = cp.tile([P, TN], bf16, name="scr")
            nc.gpsimd.memset(scr[:], 0.0)
            psd = sp.tile([P, TN], f32, name="psd")
            for _ in range(N_WARM):
                nc.tensor.matmul(psd[:], scr[:, 0:P], scr[:], start=True, stop=True)

            # Augmented low-rank activations: rows 0..15 = (x @ A)^T,
            # row 16 = 1, rows 17..127 = 1 (their bt rows are 0; memset of
            # the full tile also guarantees no garbage Inf/NaN reaches the
            # PE). o=0's base matmuls ride the same per-x-group cadence so
            # the DMA-paced startup phase retires real work.
            low = cp.tile([P, SH], bf16, name="low")
            nc.gpsimd.memset(low[:], 1.0)
            pls = [lp.tile([R, TN], f32, name="pl") for _ in range(TGROUPS)]
            ps0 = [pp.tile([P, TN], f32, name="ps") for _ in range(TGROUPS)]
            for g in range(XG):
                for k in range(g * KG, (g + 1) * KG):
                    for t in range(TGROUPS):
                        nc.tensor.matmul(
                            pls[t][:],
                            at[:, k * R : (k + 1) * R],
                            xsl(k, t),
                            start=(k == 0),
                            stop=(k == KC - 1),
                        )
                for t in range(TGROUPS):
                    for k in range(g * KG, (g + 1) * KG):
                        nc.tensor.matmul(
                            ps0[t][:],
                            w_strips[0][:, k * P : (k + 1) * P],
                            xsl(k, t),
                            start=(k == 0),
                            stop=False,
                        )
            for t in range(TGROUPS):
                nc.vector.tensor_copy(low[0:R, t * TN : (t + 1) * TN], pls[t][:])

            for o in range(O_CHUNKS):
                wk = w_strips.pop(o) if o in w_strips else load_w_strip(o)
                if o + 1 < O_CHUNKS and o >= 1 and (o + 1) not in w_strips:
                    w_strips[o + 1] = load_w_strip(o + 1)
                otile = op.tile([P, SH], f32, name="otile")
                for t in range(TGROUPS):
                    if o == 0:
                        ps = ps0[t]
                    else:
                        ps = pp.tile([P, TN], f32, name="ps")
                        for k in range(KC):
                            nc.tensor.matmul(
                                ps[:],
                                wk[:, k * P : (k + 1) * P],
                                xsl(k, t),
                                start=(k == 0),
                                stop=False,
                            )
                    nc.tensor.matmul(
                        ps[:],
                        bt[:, o * P : (o + 1) * P],
                        low[:, t * TN : (t + 1) * TN],
                        start=False,
                        stop=True,
                    )
                    nc.vector.tensor_copy(otile[:, t * TN : (t + 1) * TN], ps[:])
                    nc.sync.dma_start(
                        out=ot_d.ap()[o * P : (o + 1) * P, t * TN : (t + 1) * TN],
                        in_=otile[:, t * TN : (t + 1) * TN],
                    )
    nc.compile()
    return nc


def _get_nc():
    global _cached_nc
    if _cached_nc is None:
        _cached_nc = _build()
    return _cached_nc


def _in_maps(x, weight, bias, lora_A, lora_B):
    # W^T packed as [o_chunk, partition, k*128+c]: element (o*128+c, k*128+p)
    # of W -> wt[o, p, k*128+c]; shared by all cores.
    wt = np.ascontiguousarray(
        weight.T.reshape(KC, P, O_CHUNKS, P).transpose(2, 1, 0, 3).reshape(
            O_CHUNKS, P, KC * P
        )
    ).astype(BF16)
    maps = []
    for c in range(8):
        b, h = divmod(c, 2)
        xtc = np.ascontiguousarray(
            x[b, h * SH : (h + 1) * SH, :].T.reshape(XG, KG, P, SH)
            .transpose(0, 2, 1, 3)
            .reshape(XG, P, KG * SH)
        ).astype(BF16)
        apk = np.ascontiguousarray(
            lora_A[b].reshape(KC, P, R).transpose(1, 0, 2).reshape(P, KC * R)
        ).astype(BF16)
        baug = np.zeros((P, D_OUT), np.float32)
        baug[0:R] = lora_B[b] * SCALE
        baug[R] = bias
        maps.append({"xt": xtc, "wt": wt, "apack": apk, "baug": baug.astype(BF16)})
    return maps


def kernel(x, weight, bias, lora_A, lora_B, _trace=False, _tmpdir=None):
    x = np.asarray(x, dtype=np.float32)
    weight = np.asarray(weight, dtype=np.float32)
    bias = np.asarray(bias, dtype=np.float32)
    lora_A = np.asarray(lora_A, dtype=np.float32)
    lora_B = np.asarray(lora_B, dtype=np.float32)

    nc = _get_nc()
    maps = _in_maps(x, weight, bias, lora_A, lora_B)
    res = run_bass_kernel_spmd(
        nc, maps, list(range(8)), trace=_trace, tmpdir=_tmpdir
    )
    out = np.empty((B, S, D_OUT), np.float32)
    for c in range(8):
        b, h = divmod(c, 2)
        out[b, h * SH : (h + 1) * SH, :] = res.results[c]["ot"].T
    if _trace:
        return out, res
    return out


# revision 6
# speedup vs baseline: 1.1793x; 1.0071x over previous
"""HNLoRALinear Trainium2 kernel (bf16 v4, col-tiled LoRA).

out[b,s,o] = x[b] @ W^T + bias + SCALE * (x[b] @ A[b]) @ B[b]

Sharding: 8 cores = 4 batches x 2 sequence-halves. Each core computes
its [1024 tokens, 4096 outs] output block, TRANSPOSED on device
(outs on PSUM partitions, tokens as the moving dim). All inputs are
bf16 (PE streams 1 col/cycle for bf16 and f32r alike, but bf16 halves
DMA bytes, halves SBUF, and enables Fast Weight Load so the 128-col
LDWEIGHTS fully hides under the 512-col stream); accumulation is fp32
in PSUM.

Startup: 10 dummy matmuls on a memset scratch tile warm the PE
p-state while the first DMAs land, then o=0/o=1's base matmuls run on
the per-x-group DMA cadence so the x-load window retires real work
(4 open PSUM groups + 2 lora banks + 1 scratch = 7 of 8 banks).

The x@A low-rank pass uses PE column tiling: the [128,16] A-chunk
stationaries occupy one 32-col tile each, so 4 consecutive k-chunks
stream CONCURRENTLY on 4 XBUSes into 4 PSUM quadrants (16 quad-spans
instead of 64 serial 512-col streams). The 4 quadrant partials are
then summed on the vector engine into the bf16 `low` tile.

The LoRA correction + bias ride along as one extra matmul per output
tile, with the stationary padded to the full K=128 (rows 0..15 =
SCALE*B chunk, row 16 = bias, rest 0) so the LDWEIGHTS pull-ahead
pipeline stays intact across the tile boundary.
"""
import numpy as np
import ml_dtypes

import concourse.bass as bass  # noqa: F401  (bass must import before tile)
import concourse.mybir as mybir
import concourse.tile as tile
from concourse import bacc
from concourse.bass_utils import run_bass_kernel_spmd

# Problem shapes (hardcoded per contract).
B, S, D_IN, D_OUT, R = 4, 2048, 4096, 4096, 16
XG = 8                 # x DMA groups (separate tiles so deps are per-group)
SCALE = 32.0 / 16.0
SH = S // 2            # tokens per core
P = 128
KC = D_IN // P         # 32 contraction chunks
O_CHUNKS = D_OUT // P  # 32 output-feature chunks (PSUM partition dim)
TN = 512               # moving-dim token group width
TGROUPS = SH // TN     # 2
KG = KC // XG          # k-chunks per x group
RA = R + 1             # augmented rank (lora + bias row)
N_WARM = 10            # PE warm-up dummy matmuls
QT = 4                 # col-tiles per quad (32-col tiles)

BF16 = ml_dtypes.bfloat16

_cached_nc = None


def _build():
    bf16 = mybir.dt.bfloat16
    f32 = mybir.dt.float32
    nc = bacc.Bacc(
        "TRN2", target_bir_lowering=False, debug=False, enable_asserts=False
    )
    xt = nc.dram_tensor("xt", [XG, P, KG * SH], bf16, kind="ExternalInput")
    wt = nc.dram_tensor("wt", [O_CHUNKS, P, KC * P], bf16, kind="ExternalInput")
    apk = nc.dram_tensor("apack", [P, KC * R], bf16, kind="ExternalInput")
    bga = nc.dram_tensor("baug", [P, D_OUT], bf16, kind="ExternalInput")
    ot_d = nc.dram_tensor("ot", [D_OUT, SH], f32, kind="ExternalOutput")

    with tile.TileContext(nc) as tc:
        with (
            tc.tile_pool(name="xp", bufs=1) as xp,
            tc.tile_pool(name="wp", bufs=3) as wp,
            tc.tile_pool(name="cp", bufs=1) as cp,
            tc.tile_pool(name="op", bufs=3) as op,
            tc.tile_pool(name="pp", bufs=4, space="PSUM") as pp,
            tc.tile_pool(name="lp", bufs=2, space="PSUM") as lp,
            tc.tile_pool(name="sp", bufs=1, space="PSUM") as sp,
        ):
            at = cp.tile([P, KC * R], bf16, name="at")
            nc.sync.dma_start(out=at[:], in_=apk.ap())

            def load_w_strip(o):
                # One fully-contiguous 2D DMA per strip (host pre-packs W
                # as [o_chunk, partition, k*128+c]) -- 3D patterns cost one
                # DMA descriptor per (partition, k) and dispatch ~20x slower.
                wk = wp.tile([P, KC * P], bf16, name="wk")
                nc.sync.dma_start(out=wk[:], in_=wt.ap()[o])
                return wk

            # x^T fully resident as XG separate [128, KG, 1024] tiles (one
            # DMA each) so each matmul depends only on the chunk-group it
            # reads; interleave the first W strips so main o=0/1 can start
            # early. bt (padded lora_B+bias) is only needed at the first
            # augmented matmul, so it is enqueued after xg1.
            xgs = []
            w_strips = {}
            bt = cp.tile([P, D_OUT], bf16, name="bt")
            for g in range(XG):
                xg = xp.tile([P, KG * SH], bf16, name=f"xg{g}", tag=f"xg{g}")
                nc.sync.dma_start(out=xg[:], in_=xt.ap()[g])
                xgs.append(xg)
                if g < 2:
                    w_strips[g] = load_w_strip(g)
                if g == 1:
                    nc.sync.dma_start(out=bt[:], in_=bga.ap())

            def xsl(k, t):
                return xgs[k // KG][:, (k % KG) * SH + t * TN : (k % KG) * SH + (t + 1) * TN]

            # PE warm-up: dummy matmuls on a zeroed scratch tile with no DMA
            # dependency, so the PE ramps to full clock while at/xg0 land.
            scr = cp.tile([P, TN], bf16, name="scr")
            nc.gpsimd.memset(scr[:], 0.0)
            psd = sp.tile([P, TN], f32, name="psd")
            for _ in range(N_WARM):
                nc.tensor.matmul(psd[:], scr[:, 0:P], scr[:], start=True, stop=True)

            low = cp.tile([P, SH], bf16, name="low")
            nc.gpsimd.memset(low[:], 1.0)

            # Phase 1: o=0 and o=1 base matmuls ride the per-x-group DMA
            # cadence (4 PSUM groups stay open until their augmented matmul).
            ps_early = [
                [pp.tile([P, TN], f32, name="ps") for _ in range(TGROUPS)]
                for _ in range(2)
            ]
            for g in range(XG):
                for o in range(2):
                    for t in range(TGROUPS):
                        for k in range(g * KG, (g + 1) * KG):
                            nc.tensor.matmul(
                                ps_early[o][t][:],
                                w_strips[o][:, k * P : (k + 1) * P],
                                xsl(k, t),
                                start=(k == 0),
                                stop=False,
                            )

            # Phase 2: x@A via column tiling -- k-chunks k=4q+j stream
            # concurrently into col-tile j / PSUM quadrant j.
            pls = [lp.tile([P, TN], f32, name="pl") for _ in range(TGROUPS)]
            for t in range(TGROUPS):
                for q in range(KC // QT):
                    for j in range(QT):
                        k = QT * q + j
                        nc.tensor.matmul(
                            pls[t][32 * j : 32 * j + R, :],
                            at[:, k * R : (k + 1) * R],
                            xsl(k, t),
                            start=(q == 0),
                            stop=(q == KC // QT - 1),
                            tile_position=(0, 32 * j),
                        )

            # Phase 3: copy each quadrant partial to the SAME partitions of
            # `low` (rows 32j..32j+15). No cross-partition reduce is needed:
            # bt carries SCALE*B at those same four row positions, so the
            # augmented matmul's K=128 contraction sums the quadrants.
            for t in range(TGROUPS):
                sl = slice(t * TN, (t + 1) * TN)
                for j in range(QT):
                    nc.vector.tensor_copy(
                        low[32 * j : 32 * j + R, sl], pls[t][32 * j : 32 * j + R, :]
                    )

            # Phase 4: augmented matmuls close the early groups; then the
            # steady o-loop.
            def finish_tile(o, ps, otile, t):
                nc.tensor.matmul(
                    ps[:],
                    bt[:, o * P : (o + 1) * P],
                    low[:, t * TN : (t + 1) * TN],
                    start=False,
                    stop=True,
                )
                nc.vector.tensor_copy(otile[:, t * TN : (t + 1) * TN], ps[:])
                nc.sync.dma_start(
                    out=ot_d.ap()[o * P : (o + 1) * P, t * TN : (t + 1) * TN],
                    in_=otile[:, t * TN : (t + 1) * TN],
                )

            early_otiles = [op.tile([P, SH], f32, name="otile") for _ in range(2)]
            for o in range(2):
                for t in range(TGROUPS):
                    finish_tile(o, ps_early[o][t], early_otiles[o], t)

            w_strips[2] = load_w_strip(2)
            for o in range(2, O_CHUNKS):
                wk = w_strips.pop(o) if o in w_strips else load_w_strip(o)
                if o + 1 < O_CHUNKS and (o + 1) not in w_strips:
                    w_strips[o + 1] = load_w_strip(o + 1)
                otile = op.tile([P, SH], f32, name="otile")
                for t in range(TGROUPS):
                    ps = pp.tile([P, TN], f32, name="ps")
                    for k in range(KC):
                        nc.tensor.matmul(
                            ps[:],
                            wk[:, k * P : (k + 1) * P],
                            xsl(k, t),
                            start=(k == 0),
                            stop=False,
                        )
                    finish_tile(o, ps, otile, t)
    nc.compile()
    return nc


def _get_nc():
    global _cached_nc
    if _cached_nc is None:
        _cached_nc = _build()
    return _cached_nc


def _in_maps(x, weight, bias, lora_A, lora_B):
    # W^T packed as [o_chunk, partition, k*128+c]: element (o*128+c, k*128+p)
    # of W -> wt[o, p, k*128+c]; shared by all cores.
    wt = np.ascontiguousarray(
        weight.T.reshape(KC, P, O_CHUNKS, P).transpose(2, 1, 0, 3).reshape(
            O_CHUNKS, P, KC * P
        )
    ).astype(BF16)
    maps = []
    for c in range(8):
        b, h = divmod(c, 2)
        xtc = np.ascontiguousarray(
            x[b, h * SH : (h + 1) * SH, :].T.reshape(XG, KG, P, SH)
            .transpose(0, 2, 1, 3)
            .reshape(XG, P, KG * SH)
        ).astype(BF16)
        apk = np.ascontiguousarray(
            lora_A[b].reshape(KC, P, R).transpose(1, 0, 2).reshape(P, KC * R)
        ).astype(BF16)
        baug = np.zeros((P, D_OUT), np.float32)
        for j in range(4):
            baug[32 * j : 32 * j + R] = lora_B[b] * SCALE
        baug[R] = bias
        maps.append({"xt": xtc, "wt": wt, "apack": apk, "baug": baug.astype(BF16)})
    return maps


def kernel(x, weight, bias, lora_A, lora_B, _trace=False, _tmpdir=None):
    x = np.asarray(x, dtype=np.float32)
    weight = np.asarray(weight, dtype=np.float32)
    bias = np.asarray(bias, dtype=np.float32)
    lora_A = np.asarray(lora_A, dtype=np.float32)
    lora_B = np.asarray(lora_B, dtype=np.float32)

    nc = _get_nc()
    maps = _in_maps(x, weight, bias, lora_A, lora_B)
    res = run_bass_kernel_spmd(
        nc, maps, list(range(8)), trace=_trace, tmpdir=_tmpdir
    )
    out = np.empty((B, S, D_OUT), np.float32)
    for c in range(8):
        b, h = divmod(c, 2)
        out[b, h * SH : (h + 1) * SH, :] = res.results[c]["ot"].T
    if _trace:
        return out, res
    return out


# revision 9
# speedup vs baseline: 1.1854x; 1.0052x over previous
"""HNLoRALinear Trainium2 kernel (bf16 v4, col-tiled LoRA).

out[b,s,o] = x[b] @ W^T + bias + SCALE * (x[b] @ A[b]) @ B[b]

Sharding: 8 cores = 4 batches x 2 sequence-halves. Each core computes
its [1024 tokens, 4096 outs] output block, TRANSPOSED on device
(outs on PSUM partitions, tokens as the moving dim). All inputs are
bf16 (PE streams 1 col/cycle for bf16 and f32r alike, but bf16 halves
DMA bytes, halves SBUF, and enables Fast Weight Load so the 128-col
LDWEIGHTS fully hides under the 512-col stream); accumulation is fp32
in PSUM.

Startup: 10 dummy matmuls on a memset scratch tile warm the PE
p-state while the first DMAs land, then o=0/o=1's base matmuls run on
the per-x-group DMA cadence so the x-load window retires real work
(4 open PSUM groups + 2 lora banks + 1 scratch = 7 of 8 banks).

The x@A low-rank pass uses PE column tiling: the [128,16] A-chunk
stationaries occupy one 32-col tile each, so 4 consecutive k-chunks
stream CONCURRENTLY on 4 XBUSes into 4 PSUM quadrants (16 quad-spans
instead of 64 serial 512-col streams). The 4 quadrant partials are
then summed on the vector engine into the bf16 `low` tile.

The LoRA correction + bias ride along as one extra matmul per output
tile, with the stationary padded to the full K=128 (rows 0..15 =
SCALE*B chunk, row 16 = bias, rest 0) so the LDWEIGHTS pull-ahead
pipeline stays intact across the tile boundary.
"""
import numpy as np
import ml_dtypes

import concourse.bass as bass  # noqa: F401  (bass must import before tile)
import concourse.mybir as mybir
import concourse.tile as tile
from concourse import bacc
from concourse.bass_utils import run_bass_kernel_spmd

# Problem shapes (hardcoded per contract).
B, S, D_IN, D_OUT, R = 4, 2048, 4096, 4096, 16
XG = 8                 # x DMA groups (separate tiles so deps are per-group)
SCALE = 32.0 / 16.0
SH = S // 2            # tokens per core
P = 128
KC = D_IN // P         # 32 contraction chunks
O_CHUNKS = D_OUT // P  # 32 output-feature chunks (PSUM partition dim)
TN = 512               # moving-dim token group width
TGROUPS = SH // TN     # 2
KG = KC // XG          # k-chunks per x group
RA = R + 1             # augmented rank (lora + bias row)
N_WARM = 18            # PE warm-up dummy matmuls
QT = 4                 # col-tiles per quad (32-col tiles)

BF16 = ml_dtypes.bfloat16

_cached_nc = None


def _build():
    bf16 = mybir.dt.bfloat16
    f32 = mybir.dt.float32
    nc = bacc.Bacc(
        "TRN2", target_bir_lowering=False, debug=False, enable_asserts=False
    )
    xt = nc.dram_tensor("xt", [XG, P, KG * SH], bf16, kind="ExternalInput")
    wt = nc.dram_tensor("wt", [O_CHUNKS, P, KC * P], bf16, kind="ExternalInput")
    apk = nc.dram_tensor("apack", [P, KC * R], bf16, kind="ExternalInput")
    bga = nc.dram_tensor("baug", [P, D_OUT], bf16, kind="ExternalInput")
    ot_d = nc.dram_tensor("ot", [D_OUT, SH], f32, kind="ExternalOutput")

    with tile.TileContext(nc) as tc:
        with (
            tc.tile_pool(name="xp", bufs=1) as xp,
            tc.tile_pool(name="wp", bufs=3) as wp,
            tc.tile_pool(name="cp", bufs=1) as cp,
            tc.tile_pool(name="op", bufs=3) as op,
            tc.tile_pool(name="pp", bufs=5, space="PSUM") as pp,
            tc.tile_pool(name="lp", bufs=2, space="PSUM") as lp,
            tc.tile_pool(name="sp", bufs=1, space="PSUM") as sp,
        ):
            at = cp.tile([P, KC * R], bf16, name="at")
            nc.sync.dma_start(out=at[:], in_=apk.ap())

            def load_w_strip(o):
                # One fully-contiguous 2D DMA per strip (host pre-packs W
                # as [o_chunk, partition, k*128+c]) -- 3D patterns cost one
                # DMA descriptor per (partition, k) and dispatch ~20x slower.
                wk = wp.tile([P, KC * P], bf16, name="wk")
                nc.sync.dma_start(out=wk[:], in_=wt.ap()[o])
                return wk

            # x^T fully resident as XG separate [128, KG, 1024] tiles (one
            # DMA each) so each matmul depends only on the chunk-group it
            # reads. Dispatch order is tuned so phase 1 (o=0/o=1 base
            # matmuls, ~3.5us of PE work per x group) never starves on the
            # input queue: wk0/wk1 right after their first consumers' x
            # groups, bt (only needed by the augmented matmuls ~40us in)
            # after xg3.
            xgs = [None] * XG
            w_strips = {}
            bt = cp.tile([P, D_OUT], bf16, name="bt")

            def load_xg(g):
                xg = xp.tile([P, KG * SH], bf16, name=f"xg{g}", tag=f"xg{g}")
                nc.sync.dma_start(out=xg[:], in_=xt.ap()[g])
                xgs[g] = xg

            load_xg(0)
            w_strips[0] = load_w_strip(0)
            load_xg(1)
            w_strips[1] = load_w_strip(1)
            load_xg(2)
            load_xg(3)
            nc.sync.dma_start(out=bt[:], in_=bga.ap())
            for g in range(4, XG):
                load_xg(g)

            def xsl(k, t):
                return xgs[k // KG][:, (k % KG) * SH + t * TN : (k % KG) * SH + (t + 1) * TN]

            # PE warm-up: dummy matmuls on a zeroed scratch tile with no DMA
            # dependency, so the PE ramps to full clock while at/xg0 land.
            scr = cp.tile([P, TN], bf16, name="scr")
            nc.gpsimd.memset(scr[:], 0.0)
            psd = sp.tile([P, TN], f32, name="psd")
            for _ in range(N_WARM):
                nc.tensor.matmul(psd[:], scr[:, 0:P], scr[:], start=True, stop=True)

            low = cp.tile([P, SH], bf16, name="low")
            nc.gpsimd.memset(low[:], 1.0)

            # Phase 1: o=0 and o=1 base matmuls ride the per-x-group DMA
            # cadence (4 PSUM groups stay open until their augmented matmul).
            ps_early = [
                [pp.tile([P, TN], f32, name="ps") for _ in range(TGROUPS)]
                for _ in range(2)
            ]
            for g in range(XG):
                for o in range(2):
                    for t in range(TGROUPS):
                        for k in range(g * KG, (g + 1) * KG):
                            nc.tensor.matmul(
                                ps_early[o][t][:],
                                w_strips[o][:, k * P : (k + 1) * P],
                                xsl(k, t),
                                start=(k == 0),
                                stop=False,
                            )

            # Phase 2: x@A via column tiling -- k-chunks k=4q+j stream
            # concurrently into col-tile j / PSUM quadrant j.
            pls = [lp.tile([P, TN], f32, name="pl") for _ in range(TGROUPS)]
            for t in range(TGROUPS):
                for q in range(KC // QT):
                    for j in range(QT):
                        k = QT * q + j
                        nc.tensor.matmul(
                            pls[t][32 * j : 32 * j + R, :],
                            at[:, k * R : (k + 1) * R],
                            xsl(k, t),
                            start=(q == 0),
                            stop=(q == KC // QT - 1),
                            tile_position=(0, 32 * j),
                        )

            # Phase 3: copy each quadrant partial to the SAME partitions of
            # `low` (rows 32j..32j+15). No cross-partition reduce is needed:
            # bt carries SCALE*B at those same four row positions, so the
            # augmented matmul's K=128 contraction sums the quadrants.
            for t in range(TGROUPS):
                sl = slice(t * TN, (t + 1) * TN)
                for j in range(QT):
                    nc.vector.tensor_copy(
                        low[32 * j : 32 * j + R, sl], pls[t][32 * j : 32 * j + R, :]
                    )

            # Phase 4: augmented matmuls close the early groups; then the
            # steady o-loop.
            def finish_tile(o, ps, otile, t):
                nc.tensor.matmul(
                    ps[:],
                    bt[:, o * P : (o + 1) * P],
                    low[:, t * TN : (t + 1) * TN],
                    start=False,
                    stop=True,
                )
                nc.vector.tensor_copy(otile[:, t * TN : (t + 1) * TN], ps[:])
                nc.sync.dma_start(
                    out=ot_d.ap()[o * P : (o + 1) * P, t * TN : (t + 1) * TN],
                    in_=otile[:, t * TN : (t + 1) * TN],
                )

            early_otiles = [op.tile([P, SH], f32, name="otile") for _ in range(2)]
            for o in range(2):
                for t in range(TGROUPS):
                    finish_tile(o, ps_early[o][t], early_otiles[o], t)

            w_strips[2] = load_w_strip(2)
            for o in range(2, O_CHUNKS):
                wk = w_strips.pop(o) if o in w_strips else load_w_strip(o)
                if o + 1 < O_CHUNKS and (o + 1) not in w_strips:
                    w_strips[o + 1] = load_w_strip(o + 1)
                otile = op.tile([P, SH], f32, name="otile")
                for t in range(TGROUPS):
                    ps = pp.tile([P, TN], f32, name="ps")
                    for k in range(KC):
                        nc.tensor.matmul(
                            ps[:],
                            wk[:, k * P : (k + 1) * P],
                            xsl(k, t),
                            start=(k == 0),
                            stop=False,
                        )
                    finish_tile(o, ps, otile, t)
    nc.compile()
    return nc


def _get_nc():
    global _cached_nc
    if _cached_nc is None:
        _cached_nc = _build()
    return _cached_nc


def _in_maps(x, weight, bias, lora_A, lora_B):
    # W^T packed as [o_chunk, partition, k*128+c]: element (o*128+c, k*128+p)
    # of W -> wt[o, p, k*128+c]; shared by all cores.
    wt = np.ascontiguousarray(
        weight.T.reshape(KC, P, O_CHUNKS, P).transpose(2, 1, 0, 3).reshape(
            O_CHUNKS, P, KC * P
        )
    ).astype(BF16)
    maps = []
    for c in range(8):
        b, h = divmod(c, 2)
        xtc = np.ascontiguousarray(
            x[b, h * SH : (h + 1) * SH, :].T.reshape(XG, KG, P, SH)
            .transpose(0, 2, 1, 3)
            .reshape(XG, P, KG * SH)
        ).astype(BF16)
        apk = np.ascontiguousarray(
            lora_A[b].reshape(KC, P, R).transpose(1, 0, 2).reshape(P, KC * R)
        ).astype(BF16)
        baug = np.zeros((P, D_OUT), np.float32)
        for j in range(4):
            baug[32 * j : 32 * j + R] = lora_B[b] * SCALE
        baug[R] = bias
        maps.append({"xt": xtc, "wt": wt, "apack": apk, "baug": baug.astype(BF16)})
    return maps


def kernel(x, weight, bias, lora_A, lora_B, _trace=False, _tmpdir=None):
    x = np.asarray(x, dtype=np.float32)
    weight = np.asarray(weight, dtype=np.float32)
    bias = np.asarray(bias, dtype=np.float32)
    lora_A = np.asarray(lora_A, dtype=np.float32)
    lora_B = np.asarray(lora_B, dtype=np.float32)

    nc = _get_nc()
    maps = _in_maps(x, weight, bias, lora_A, lora_B)
    res = run_bass_kernel_spmd(
        nc, maps, list(range(8)), trace=_trace, tmpdir=_tmpdir
    )
    out = np.empty((B, S, D_OUT), np.float32)
    for c in range(8):
        b, h = divmod(c, 2)
        out[b, h * SH : (h + 1) * SH, :] = res.results[c]["ot"].T
    if _trace:
        return out, res
    return out
